# revision 31
# baseline (speedup 1.0000x reference)
"""Trainium2 Bass kernel for nn_MultiHeadAttention_77833397338310.

Computes (qh, attn) of the reference multi-head attention:
  qh   = (q @ W_q.T) reshaped to [B, H, Lq, 64]
  attn = softmax(mask_fill(qh/TEMP @ kh.T))   [B, H, Lq, Lk]
(The reference's v/o/fc/layernorm path is dead code - jit DCEs it.)

Sharding: 8 cores; core i handles batch b = i//4 and head pair (2*(i%4), 2*(i%4)+1).
Each core loads q[b], k[b] once, PE-transposes them, projects per-head
qh^T/kh^T (head dim on partitions), then computes scores with the mask bias
fused into the matmul as a rank-1 term (ones row in lhsT x bias row in rhs).
Softmax: ScalarE exp with free accumulated row sums, VectorE reciprocal +
per-partition scale, DMA out.
"""

import numpy as np

B = 2
L = 2048
D = 512
H = 8
DH = 64
TEMP = float(DH) ** 0.5
P = 128
NT = L // P   # 16 row tiles
NC = D // P   # 4 contraction chunks
NB = L // 512  # 4 free chunks of 512

# matmul operand dtype per stage: "f32" (exact, 4 cyc/row) or "f32r" (1 cyc/row)
# "f16" for scores: normal PE mode (keeps the HAM clock gate warm), 1 cyc/row
DT_SCORES = "f16"    # qhT/khT tiles + scores matmul
DT_K = "f16"         # kT + wk tiles (khT projection path): real-mode PE, HAM-visible
DT_Q = "f32r"        # qT + wq/wqo tiles; "f32" = exact qh output but 2x PE cost

# run options (test harness may override; grading uses defaults)
run_opts = {"trace": False, "tmpdir": None, "trace_cores": None}
_cache = {}


def _build():
    import concourse.bacc as bacc
    import concourse.bass as bass
    import concourse.mybir as mybir
    import concourse.tile as tile
    from concourse import masks
    from concourse.bass import ts

    f32 = mybir.dt.float32
    f32r = mybir.dt.float32r
    mdt_s = {"f32r": f32r, "f16": mybir.dt.float16, "f32": f32}[DT_SCORES]
    mdt_k = {"f32r": f32r, "f16": mybir.dt.float16, "f32": f32}[DT_K]
    mdt_q = f32r if DT_Q == "f32r" else f32

    nc = bacc.Bacc("TRN2", target_bir_lowering=False, debug=False)

    q_d = nc.dram_tensor("q", [L, D], f32, kind="ExternalInput")
    k_d = nc.dram_tensor("k", [L, D], f32, kind="ExternalInput")
    wq_d = nc.dram_tensor("wq", [2, NC, P, DH], mdt_q, kind="ExternalInput")
    wk_d = nc.dram_tensor("wk", [2, NC, P, DH], mdt_k, kind="ExternalInput")
    wqo_d = nc.dram_tensor("wqo", [NC, P, 2 * DH], mdt_q, kind="ExternalInput")
    bias_d = nc.dram_tensor("bias", [2, L], mdt_s, kind="ExternalInput")  # row0=mask bias, row1=ones
    attn_d = nc.dram_tensor("attn", [2, L, L], f32, kind="ExternalOutput")
    # qh in SBUF-native layout [p, t, h*64+m]; host rearranges (contiguous DMA)
    qh_d = nc.dram_tensor("qh", [P, NT, 2 * DH], f32, kind="ExternalOutput")

    with tile.TileContext(nc) as tc:
        with (
            tc.tile_pool(name="const", bufs=1) as constp,
            tc.tile_pool(name="nat", bufs=4) as natp,
            tc.tile_pool(name="tposed", bufs=1) as tpp,
            tc.tile_pool(name="headp", bufs=1) as headp,
            tc.tile_pool(name="attn", bufs=4) as attnp,
            tc.tile_pool(name="psum", bufs=2, space=bass.MemorySpace.PSUM) as psp,
        ):
            ident = constp.tile([P, P], f32, name="ident", tag="ident")
            masks.make_identity(nc, ident[:])

            # HAM warm-up: ~3.4us of back-to-back zero matmuls while the PE
            # would otherwise idle waiting for the first input DMA, so the
            # clock gate is already at 2.4GHz when the transposes start.
            zerob = constp.tile([P, 512], mybir.dt.bfloat16, name="zerob", tag="zerob")
            nc.gpsimd.memset(zerob[:], 0.0)
            wtile = psp.tile([P, 512], f32, name="pw", tag="ps")
            for _ in range(8):
                nc.tensor.matmul(
                    wtile[:], zerob[:, 0:P], zerob[:], start=True, stop=True
                )

            wq_sb = [
                constp.tile([P, NC, DH], mdt_q, name=f"wq{h}", tag=f"wq{h}")
                for h in range(2)
            ]
            wk_sb = [
                constp.tile([P, NC, DH], mdt_k, name=f"wk{h}", tag=f"wk{h}")
                for h in range(2)
            ]
            wqo_sb = constp.tile([P, NC, 2 * DH], mdt_q, name="wqo", tag="wqo")
            for h in range(2):
                nc.gpsimd.dma_start(wq_sb[h][:], wq_d[h].rearrange("c p m -> p c m"))
                nc.gpsimd.dma_start(wk_sb[h][:], wk_d[h].rearrange("c p m -> p c m"))
            nc.gpsimd.dma_start(wqo_sb[:], wqo_d[:].rearrange("c p m -> p c m"))

            sums = [
                constp.tile([P, NT], f32, name=f"sums{h}", tag=f"sums{h}")
                for h in range(2)
            ]
            recip = [
                constp.tile([P, NT], f32, name=f"recip{h}", tag=f"recip{h}")
                for h in range(2)
            ]

            # ---- load k,q natural; PE-transpose into kT/qT [128, c, 2048] ----
            kT = tpp.tile([P, NC, L], mdt_k, name="kT", tag="kT")
            qT = tpp.tile([P, NC, L], mdt_q, name="qT", tag="qT")

            # per-head projections: qhT_ext/khT_ext [65, 2048]
            # row 64 of qhT_ext = ones; row 64 of khT_ext = mask bias
            qhT = [
                headp.tile([DH + 1, L], mdt_s, name=f"qhT{h}", tag=f"qhT{h}")
                for h in range(2)
            ]
            khT = [
                headp.tile([DH + 1, L], mdt_s, name=f"khT{h}", tag=f"khT{h}")
                for h in range(2)
            ]
            qhs = constp.tile([P, NT, 2 * DH], f32, name="qhs", tag="qhs")
            for h in range(2):
                nc.gpsimd.dma_start(khT[h][DH : DH + 1, :], bias_d[0:1, :])
                nc.gpsimd.dma_start(qhT[h][DH : DH + 1, :], bias_d[1:2, :])

            NQ = 4  # load quarters (1MB each)
            TQ = NT // NQ

            def load_nat(src_r, quart, eng=None):
                nat = natp.tile([P, TQ, D], f32, name="nat", tag="nat")
                (eng or nc.sync).dma_start(nat[:], src_r[:, ts(quart, TQ), :])
                return nat

            def transp_c(nat, dst, quart, c):
                pt = psp.tile([P, TQ * P], f32, name="pst", tag="ps")
                for t in range(TQ):
                    nc.tensor.transpose(
                        pt[:, ts(t, P)], nat[:, t, ts(c, P)], ident[:]
                    )
                nc.vector.tensor_copy(dst[:, c, ts(quart, TQ * P)], pt[:])

            def project_chunk(h, w_sb, src, dst, n):
                pp = psp.tile([DH, 512], f32, name="psp", tag="ps")
                for c in range(NC):
                    nc.tensor.matmul(
                        pp[:],
                        w_sb[h][:, c, :],
                        src[:, c, ts(n, 512)],
                        start=(c == 0),
                        stop=(c == NC - 1),
                    )
                nc.vector.tensor_copy(dst[h][0:DH, ts(n, 512)], pp[:])

            def qhout_t(t):
                po = psp.tile([P, 2 * DH], f32, name="pso", tag="ps")
                for c in range(NC):
                    nc.tensor.matmul(
                        po[:],
                        qT[:, c, ts(t, P)],
                        wqo_sb[:, c, :],
                        start=(c == 0),
                        stop=(c == NC - 1),
                    )
                nc.scalar.copy(qhs[:, t, :], po[:])

            def scores_pair(h, tp, split_dma=False):
                at = attnp.tile([P, 2, L], f32, name="at", tag="attn")
                for u in range(2):
                    t = 2 * tp + u
                    sp = psp.tile([P, L], f32, name="pss", tag="ps")
                    for j in range(NB):
                        nc.tensor.matmul(
                            sp[:, ts(j, 512)],
                            qhT[h][:, ts(t, P)],
                            khT[h][:, ts(j, 512)],
                            start=True,
                            stop=True,
                        )
                    nc.scalar.activation(
                        at[:, u, :],
                        sp[:],
                        mybir.ActivationFunctionType.Exp,
                        accum_out=sums[h][:, t : t + 1],
                    )
                    nc.vector.reciprocal(
                        recip[h][:, t : t + 1], sums[h][:, t : t + 1]
                    )
                    nc.vector.tensor_scalar_mul(
                        at[:, u, :], at[:, u, :], recip[h][:, t : t + 1]
                    )
                    if split_dma:
                        nc.sync.dma_start(attn_d[h, ts(t, P), :], at[:, u, :])
                if not split_dma:
                    nc.sync.dma_start(
                        attn_d[h, ts(tp, 2 * P), :].rearrange(
                            "(u p) c -> p u c", p=P
                        ),
                        at[:],
                    )


            k_r = k_d[:].rearrange("(t p) d -> p t d", p=P)
            q_r = q_d[:].rearrange("(t p) d -> p t d", p=P)

            # k prefix: transposes + both heads' khT per quarter
            kn0a = natp.tile([P, TQ // 2, D], f32, name="nat", tag="nat")
            nc.sync.dma_start(kn0a[:], k_r[:, 0 : TQ // 2, :])
            kn0b = natp.tile([P, TQ // 2, D], f32, name="nat", tag="nat")
            nc.sync.dma_start(kn0b[:], k_r[:, TQ // 2 : TQ, :])
            knat = [None, load_nat(k_r, 1), load_nat(k_r, 2), None]
            qnat = [load_nat(q_r, 0, nc.scalar), None, None, None]
            knat[3] = load_nat(k_r, 3)
            for c in range(NC):
                pt = psp.tile([P, TQ * P], f32, name="pst", tag="ps")
                for t in range(TQ // 2):
                    nc.tensor.transpose(
                        pt[:, ts(t, P)], kn0a[:, t, ts(c, P)], ident[:]
                    )
                for t in range(TQ // 2):
                    nc.tensor.transpose(
                        pt[:, ts(TQ // 2 + t, P)], kn0b[:, t, ts(c, P)], ident[:]
                    )
                nc.vector.tensor_copy(kT[:, c, 0 : TQ * P], pt[:])
            project_chunk(0, wk_sb, kT, khT, 0)
            for quart in (1, 2):
                for c in range(NC):
                    transp_c(knat[quart], kT, quart, c)
                project_chunk(0, wk_sb, kT, khT, quart)

            def prework(quart):
                for c in range(NC):
                    transp_c(qnat[quart], qT, quart, c)
                project_chunk(0, wq_sb, qT, qhT, quart)
                project_chunk(1, wq_sb, qT, qhT, quart)

            # q0 prework runs while k quarter 3's DMA is still in flight
            prework(0)
            for c in range(NC):
                transp_c(knat[3], kT, 3, c)
            project_chunk(0, wk_sb, kT, khT, 3)
            for quart in range(NQ):
                if quart + 1 < NQ:
                    qnat[quart + 1] = load_nat(q_r, quart + 1)
                if quart == 0:
                    # khT head-1 deferred out of the prefix, spread as two
                    # even-parity blocks between the h0 pairs
                    scores_pair(0, 0, split_dma=True)
                    project_chunk(1, wk_sb, kT, khT, 0)
                    project_chunk(1, wk_sb, kT, khT, 1)
                    scores_pair(0, 1)
                    project_chunk(1, wk_sb, kT, khT, 2)
                    project_chunk(1, wk_sb, kT, khT, 3)
                    scores_pair(1, 0)
                    scores_pair(1, 1)
                else:
                    for h, tp in (
                        (0, 2 * quart),
                        (1, 2 * quart),
                        (0, 2 * quart + 1),
                        (1, 2 * quart + 1),
                    ):
                        scores_pair(
                            h,
                            tp,
                            split_dma=(quart == NQ - 1 and h == 1 and tp % 2 == 1),
                        )
                if quart + 1 < NQ:
                    prework(quart + 1)

            # qh-out projection at the tail: its PE time hides inside the
            # final attn-DMA drain + kernel epilogue
            for t in range(NT):
                qhout_t(t)
            nc.sync.dma_start(qh_d[:], qhs[:])

    nc.compile()
    return nc


def kernel(q, k, v, mask, W_q, W_k, W_v, W_fc, ln_g, ln_b):
    from concourse.bass_utils import run_bass_kernel_spmd

    if "nc" not in _cache:
        _cache["nc"] = _build()
    nc = _cache["nc"]

    q = np.asarray(q, np.float32)
    k = np.asarray(k, np.float32)
    mask = np.asarray(mask)
    W_q = np.asarray(W_q, np.float32)
    W_k = np.asarray(W_k, np.float32)

    WqT = np.ascontiguousarray(W_q.T)  # [D, H*DH]
    WkT = np.ascontiguousarray(W_k.T)

    in_maps = []
    for core in range(8):
        b, pair = divmod(core, 4)
        h0 = 2 * pair
        wq = np.ascontiguousarray(
            (WqT[:, h0 * DH : (h0 + 2) * DH] / TEMP)
            .reshape(NC, P, 2, DH)
            .transpose(2, 0, 1, 3)
        )  # [2, NC, P, DH] scaled for scores
        wk = np.ascontiguousarray(
            WkT[:, h0 * DH : (h0 + 2) * DH]
            .reshape(NC, P, 2, DH)
            .transpose(2, 0, 1, 3)
        ).astype(np.float16 if DT_K == "f16" else np.float32)
        wqo = np.ascontiguousarray(
            WqT[:, h0 * DH : (h0 + 2) * DH].reshape(NC, P, 2 * DH)
        )  # unscaled, both heads packed
        bias = np.stack(
            [
                np.where(mask[b, 0] == 0, np.float32(-60000.0), np.float32(0.0)),
                np.ones(L, np.float32),
            ]
        ).astype(np.float16 if DT_SCORES == "f16" else np.float32)
        in_maps.append(
            {
                "q": np.ascontiguousarray(q[b]),
                "k": np.ascontiguousarray(k[b]),
                "wq": wq,
                "wk": wk,
                "wqo": wqo,
                "bias": bias,
            }
        )

    res = run_bass_kernel_spmd(
        nc,
        in_maps,
        core_ids=list(range(8)),
        trace=run_opts["trace"],
        tmpdir=run_opts["tmpdir"],
        trace_cores=run_opts["trace_cores"],
    )
    _cache["last_result"] = res

    qh_full = np.empty((B, H, L, DH), np.float32)
    attn_full = np.empty((B, H, L, L), np.float32)
    for core in range(8):
        b, pair = divmod(core, 4)
        h0 = 2 * pair
        r = res.results[core]
        qh_full[b, h0 : h0 + 2] = (
            r["qh"].reshape(P, NT, 2, DH).transpose(2, 1, 0, 3).reshape(2, L, DH)
        )
        attn_full[b, h0 : h0 + 2] = r["attn"]
    return qh_full, attn_full


# revision 32
# speedup vs baseline: 1.0336x; 1.0336x over previous
"""Trainium2 Bass kernel for nn_MultiHeadAttention_77833397338310.

Computes (qh, attn) of the reference multi-head attention:
  qh   = (q @ W_q.T) reshaped to [B, H, Lq, 64]
  attn = softmax(mask_fill(qh/TEMP @ kh.T))   [B, H, Lq, Lk]
(The reference's v/o/fc/layernorm path is dead code - jit DCEs it.)

Sharding: 8 cores; core i handles batch b = i//4 and head pair (2*(i%4), 2*(i%4)+1).
Each core loads q[b], k[b] once, PE-transposes them, projects per-head
qh^T/kh^T (head dim on partitions), then computes scores with the mask bias
fused into the matmul as a rank-1 term (ones row in lhsT x bias row in rhs).
Softmax: ScalarE exp with free accumulated row sums, VectorE reciprocal +
per-partition scale, DMA out.
"""

import numpy as np

B = 2
L = 2048
D = 512
H = 8
DH = 64
TEMP = float(DH) ** 0.5
P = 128
NT = L // P   # 16 row tiles
NC = D // P   # 4 contraction chunks
NB = L // 512  # 4 free chunks of 512

# matmul operand dtype per stage: "f32" (exact, 4 cyc/row) or "f32r" (1 cyc/row)
# "f16" for scores: normal PE mode (keeps the HAM clock gate warm), 1 cyc/row
DT_SCORES = "f16"    # qhT/khT tiles + scores matmul
DT_K = "f32r"        # kT + wk tiles (khT projection path)
DT_Q = "f32r"        # qT + wq/wqo tiles; "f32" = exact qh output but 2x PE cost

# run options (test harness may override; grading uses defaults)
run_opts = {"trace": False, "tmpdir": None, "trace_cores": None}
_cache = {}


def _build():
    import concourse.bacc as bacc
    import concourse.bass as bass
    import concourse.mybir as mybir
    import concourse.tile as tile
    from concourse import masks
    from concourse.bass import ts

    f32 = mybir.dt.float32
    f32r = mybir.dt.float32r
    mdt_s = {"f32r": f32r, "f16": mybir.dt.float16, "f32": f32}[DT_SCORES]
    mdt_k = f32r if DT_K == "f32r" else f32
    mdt_q = f32r if DT_Q == "f32r" else f32

    nc = bacc.Bacc("TRN2", target_bir_lowering=False, debug=False)

    q_d = nc.dram_tensor("q", [L, D], f32, kind="ExternalInput")
    k_d = nc.dram_tensor("k", [L, D], f32, kind="ExternalInput")
    wq_d = nc.dram_tensor("wq", [2, NC, P, DH], mdt_q, kind="ExternalInput")
    wk_d = nc.dram_tensor("wk", [2, NC, P, DH], mdt_k, kind="ExternalInput")
    wqo_d = nc.dram_tensor("wqo", [NC, P, 2 * DH], mdt_q, kind="ExternalInput")
    bias_d = nc.dram_tensor("bias", [2, L], mdt_s, kind="ExternalInput")  # row0=mask bias, row1=ones
    attn_d = nc.dram_tensor("attn", [2, L, L], f32, kind="ExternalOutput")
    # qh in SBUF-native layout [p, t, h*64+m]; host rearranges (contiguous DMA)
    qh_d = nc.dram_tensor("qh", [P, NT, 2 * DH], f32, kind="ExternalOutput")

    with tile.TileContext(nc) as tc:
        with (
            tc.tile_pool(name="const", bufs=1) as constp,
            tc.tile_pool(name="nat", bufs=4) as natp,
            tc.tile_pool(name="tposed", bufs=1) as tpp,
            tc.tile_pool(name="headp", bufs=1) as headp,
            tc.tile_pool(name="attn", bufs=4) as attnp,
            tc.tile_pool(name="psum", bufs=2, space=bass.MemorySpace.PSUM) as psp,
        ):
            ident = constp.tile([P, P], f32, name="ident", tag="ident")
            masks.make_identity(nc, ident[:])

            # HAM warm-up: ~3.4us of back-to-back zero matmuls while the PE
            # would otherwise idle waiting for the first input DMA, so the
            # clock gate is already at 2.4GHz when the transposes start.
            zerob = constp.tile([P, 512], mybir.dt.bfloat16, name="zerob", tag="zerob")
            nc.gpsimd.memset(zerob[:], 0.0)
            wtile = psp.tile([P, 512], f32, name="pw", tag="ps")
            for _ in range(8):
                nc.tensor.matmul(
                    wtile[:], zerob[:, 0:P], zerob[:], start=True, stop=True
                )

            wq_sb = [
                constp.tile([P, NC, DH], mdt_q, name=f"wq{h}", tag=f"wq{h}")
                for h in range(2)
            ]
            wk_sb = [
                constp.tile([P, NC, DH], mdt_k, name=f"wk{h}", tag=f"wk{h}")
                for h in range(2)
            ]
            wqo_sb = constp.tile([P, NC, 2 * DH], mdt_q, name="wqo", tag="wqo")
            for h in range(2):
                nc.gpsimd.dma_start(wq_sb[h][:], wq_d[h].rearrange("c p m -> p c m"))
                nc.gpsimd.dma_start(wk_sb[h][:], wk_d[h].rearrange("c p m -> p c m"))
            nc.gpsimd.dma_start(wqo_sb[:], wqo_d[:].rearrange("c p m -> p c m"))

            sums = [
                constp.tile([P, NT], f32, name=f"sums{h}", tag=f"sums{h}")
                for h in range(2)
            ]
            recip = [
                constp.tile([P, NT], f32, name=f"recip{h}", tag=f"recip{h}")
                for h in range(2)
            ]

            # ---- load k,q natural; PE-transpose into kT/qT [128, c, 2048] ----
            kT = tpp.tile([P, NC, L], mdt_k, name="kT", tag="kT")
            qT = tpp.tile([P, NC, L], mdt_q, name="qT", tag="qT")

            # per-head projections: qhT_ext/khT_ext [65, 2048]
            # row 64 of qhT_ext = ones; row 64 of khT_ext = mask bias
            qhT = [
                headp.tile([DH + 1, L], mdt_s, name=f"qhT{h}", tag=f"qhT{h}")
                for h in range(2)
            ]
            khT = [
                headp.tile([DH + 1, L], mdt_s, name=f"khT{h}", tag=f"khT{h}")
                for h in range(2)
            ]
            qhs = constp.tile([P, NT, 2 * DH], f32, name="qhs", tag="qhs")
            for h in range(2):
                nc.gpsimd.dma_start(khT[h][DH : DH + 1, :], bias_d[0:1, :])
                nc.gpsimd.dma_start(qhT[h][DH : DH + 1, :], bias_d[1:2, :])

            NQ = 4  # load quarters (1MB each)
            TQ = NT // NQ

            def load_nat(src_r, quart, eng=None):
                nat = natp.tile([P, TQ, D], f32, name="nat", tag="nat")
                (eng or nc.sync).dma_start(nat[:], src_r[:, ts(quart, TQ), :])
                return nat

            def transp_c(nat, dst, quart, c):
                pt = psp.tile([P, TQ * P], f32, name="pst", tag="ps")
                for t in range(TQ):
                    nc.tensor.transpose(
                        pt[:, ts(t, P)], nat[:, t, ts(c, P)], ident[:]
                    )
                nc.vector.tensor_copy(dst[:, c, ts(quart, TQ * P)], pt[:])

            def project_chunk(h, w_sb, src, dst, n):
                pp = psp.tile([DH, 512], f32, name="psp", tag="ps")
                for c in range(NC):
                    nc.tensor.matmul(
                        pp[:],
                        w_sb[h][:, c, :],
                        src[:, c, ts(n, 512)],
                        start=(c == 0),
                        stop=(c == NC - 1),
                    )
                nc.vector.tensor_copy(dst[h][0:DH, ts(n, 512)], pp[:])

            def qhout_t(t):
                po = psp.tile([P, 2 * DH], f32, name="pso", tag="ps")
                for c in range(NC):
                    nc.tensor.matmul(
                        po[:],
                        qT[:, c, ts(t, P)],
                        wqo_sb[:, c, :],
                        start=(c == 0),
                        stop=(c == NC - 1),
                    )
                nc.scalar.copy(qhs[:, t, :], po[:])

            def scores_pair(h, tp, split_dma=False):
                at = attnp.tile([P, 2, L], f32, name="at", tag="attn")
                for u in range(2):
                    t = 2 * tp + u
                    sp = psp.tile([P, L], f32, name="pss", tag="ps")
                    for j in range(NB):
                        nc.tensor.matmul(
                            sp[:, ts(j, 512)],
                            qhT[h][:, ts(t, P)],
                            khT[h][:, ts(j, 512)],
                            start=True,
                            stop=True,
                        )
                    nc.scalar.activation(
                        at[:, u, :],
                        sp[:],
                        mybir.ActivationFunctionType.Exp,
                        accum_out=sums[h][:, t : t + 1],
                    )
                    nc.vector.reciprocal(
                        recip[h][:, t : t + 1], sums[h][:, t : t + 1]
                    )
                    nc.vector.tensor_scalar_mul(
                        at[:, u, :], at[:, u, :], recip[h][:, t : t + 1]
                    )
                    if split_dma:
                        nc.sync.dma_start(attn_d[h, ts(t, P), :], at[:, u, :])
                if not split_dma:
                    nc.sync.dma_start(
                        attn_d[h, ts(tp, 2 * P), :].rearrange(
                            "(u p) c -> p u c", p=P
                        ),
                        at[:],
                    )


            k_r = k_d[:].rearrange("(t p) d -> p t d", p=P)
            q_r = q_d[:].rearrange("(t p) d -> p t d", p=P)

            # k prefix: transposes + both heads' khT per quarter
            kn0a = natp.tile([P, TQ // 2, D], f32, name="nat", tag="nat")
            nc.sync.dma_start(kn0a[:], k_r[:, 0 : TQ // 2, :])
            kn0b = natp.tile([P, TQ // 2, D], f32, name="nat", tag="nat")
            nc.sync.dma_start(kn0b[:], k_r[:, TQ // 2 : TQ, :])
            knat = [None, load_nat(k_r, 1), load_nat(k_r, 2), None]
            qnat = [load_nat(q_r, 0, nc.scalar), None, None, None]
            knat[3] = load_nat(k_r, 3)
            for c in range(NC):
                pt = psp.tile([P, TQ * P], f32, name="pst", tag="ps")
                for t in range(TQ // 2):
                    nc.tensor.transpose(
                        pt[:, ts(t, P)], kn0a[:, t, ts(c, P)], ident[:]
                    )
                for t in range(TQ // 2):
                    nc.tensor.transpose(
                        pt[:, ts(TQ // 2 + t, P)], kn0b[:, t, ts(c, P)], ident[:]
                    )
                nc.vector.tensor_copy(kT[:, c, 0 : TQ * P], pt[:])
            project_chunk(0, wk_sb, kT, khT, 0)
            for quart in (1, 2):
                for c in range(NC):
                    transp_c(knat[quart], kT, quart, c)
                project_chunk(0, wk_sb, kT, khT, quart)

            def prework(quart):
                for c in range(NC):
                    transp_c(qnat[quart], qT, quart, c)
                project_chunk(0, wq_sb, qT, qhT, quart)
                project_chunk(1, wq_sb, qT, qhT, quart)

            # q0 prework runs while k quarter 3's DMA is still in flight
            prework(0)
            for c in range(NC):
                transp_c(knat[3], kT, 3, c)
            project_chunk(0, wk_sb, kT, khT, 3)
            for quart in range(NQ):
                if quart + 1 < NQ:
                    qnat[quart + 1] = load_nat(q_r, quart + 1)
                if quart == 0:
                    # khT head-1 deferred out of the prefix, spread as two
                    # even-parity blocks between the h0 pairs
                    scores_pair(0, 0, split_dma=True)
                    project_chunk(1, wk_sb, kT, khT, 0)
                    project_chunk(1, wk_sb, kT, khT, 1)
                    scores_pair(0, 1)
                    project_chunk(1, wk_sb, kT, khT, 2)
                    project_chunk(1, wk_sb, kT, khT, 3)
                    scores_pair(1, 0)
                    scores_pair(1, 1)
                else:
                    for h, tp in (
                        (0, 2 * quart),
                        (1, 2 * quart),
                        (0, 2 * quart + 1),
                        (1, 2 * quart + 1),
                    ):
                        scores_pair(
                            h,
                            tp,
                            split_dma=(quart == NQ - 1 and h == 1 and tp % 2 == 1),
                        )
                if quart + 1 < NQ:
                    prework(quart + 1)

            # qh-out projection at the tail: its PE time hides inside the
            # final attn-DMA drain + kernel epilogue
            for t in range(NT):
                qhout_t(t)
            nc.sync.dma_start(qh_d[:], qhs[:])

    nc.compile()
    return nc


def kernel(q, k, v, mask, W_q, W_k, W_v, W_fc, ln_g, ln_b):
    from concourse.bass_utils import run_bass_kernel_spmd

    if "nc" not in _cache:
        _cache["nc"] = _build()
    nc = _cache["nc"]

    q = np.asarray(q, np.float32)
    k = np.asarray(k, np.float32)
    mask = np.asarray(mask)
    W_q = np.asarray(W_q, np.float32)
    W_k = np.asarray(W_k, np.float32)

    WqT = np.ascontiguousarray(W_q.T)  # [D, H*DH]
    WkT = np.ascontiguousarray(W_k.T)

    in_maps = []
    for core in range(8):
        b, pair = divmod(core, 4)
        h0 = 2 * pair
        wq = np.ascontiguousarray(
            (WqT[:, h0 * DH : (h0 + 2) * DH] / TEMP)
            .reshape(NC, P, 2, DH)
            .transpose(2, 0, 1, 3)
        )  # [2, NC, P, DH] scaled for scores
        wk = np.ascontiguousarray(
            WkT[:, h0 * DH : (h0 + 2) * DH]
            .reshape(NC, P, 2, DH)
            .transpose(2, 0, 1, 3)
        )
        wqo = np.ascontiguousarray(
            WqT[:, h0 * DH : (h0 + 2) * DH].reshape(NC, P, 2 * DH)
        )  # unscaled, both heads packed
        bias = np.stack(
            [
                np.where(mask[b, 0] == 0, np.float32(-60000.0), np.float32(0.0)),
                np.ones(L, np.float32),
            ]
        ).astype(np.float16 if DT_SCORES == "f16" else np.float32)
        in_maps.append(
            {
                "q": np.ascontiguousarray(q[b]),
                "k": np.ascontiguousarray(k[b]),
                "wq": wq,
                "wk": wk,
                "wqo": wqo,
                "bias": bias,
            }
        )

    res = run_bass_kernel_spmd(
        nc,
        in_maps,
        core_ids=list(range(8)),
        trace=run_opts["trace"],
        tmpdir=run_opts["tmpdir"],
        trace_cores=run_opts["trace_cores"],
    )
    _cache["last_result"] = res

    qh_full = np.empty((B, H, L, DH), np.float32)
    attn_full = np.empty((B, H, L, L), np.float32)
    for core in range(8):
        b, pair = divmod(core, 4)
        h0 = 2 * pair
        r = res.results[core]
        qh_full[b, h0 : h0 + 2] = (
            r["qh"].reshape(P, NT, 2, DH).transpose(2, 1, 0, 3).reshape(2, L, DH)
        )
        attn_full[b, h0 : h0 + 2] = r["attn"]
    return qh_full, attn_full
